# revision 21
# baseline (speedup 1.0000x reference)
"""GCN block (3-hop symmetric-normalized propagation + LN/FFN/residual) on 8 trn2 cores.

v4: identity-slot gather (no one-hots, no scatter matmuls).
  - Table per hop: [65536, 64] fp32 (256B rows) in DRAM. Signed int16 gather
    indices with the base biased to row 32768 address all 65536 rows
    (idx = row - 32768; the Q7 address math sign-extends correctly).
  - Nodes sharded 8 ways; per core, dst nodes are degree-sorted (desc) into
    64 blocks of 128. Block b gets W[b] = max in-block degree columns
    (shared schedule = max over cores). Edge e of dst (b,p) lands at
    gather slot [partition p, column CB[b]+j] -- directly in destination
    position. Pad slots point at row 65535 (idx +32767, also guarantees the
    per-call idx tail is non-negative so the uCode trim never fires).
  - Aggregation per block = ONE strided tensor_reduce over the block's
    columns (DVE), then a rank-1 pad correction (scalar_tensor_tensor with
    per-partition pad counts x table row 65535) and the norm scale on the
    scalar engine. GPSIMD does nothing but desc-gen, on 4 SWDGE queues.
  - Hop-1 table = norm*features, host-computed. Hops 2/3 tables via
    AllGather of each core's scaled block results (fp32, 2MB/core).
  - LN + FFN fp32, interleaved per 8-block group under the hop-3 stream.
"""
import sys
sys.path.insert(0, '/opt/trn_rl_repo')
import os
import numpy as np

NC = 8          # cores
P = 128         # partitions
D = 64          # feature dim
HOPS = 3
LN_EPS = 1e-5
MAXCOLS = 28    # max gather columns per call (ring-safe; small calls pipeline better)
PADVAL = 32767  # pad idx -> table row 65535

_CACHE = {}


def _preprocess(N, edge_src, edge_dst):
    NPC = N // NC          # nodes per core (8192)
    NB = NPC // P          # dst blocks per core (64)

    deg = np.bincount(edge_dst, minlength=N).astype(np.int64)

    NCHUNK = 4                                 # AllGather chunks per hop
    BPC = NB // NCHUNK                         # blocks per chunk (16)
    order = np.empty((NC, NPC), np.int64)      # position s -> local node
    pos_of = np.empty((NC, NPC), np.int64)     # local node -> position s
    trow = np.empty(N, np.int64)               # node -> table row
    degs_blk = np.empty((NC, NB, P), np.int64)
    for k in range(NC):
        d_loc = deg[k * NPC:(k + 1) * NPC]
        o = np.argsort(-d_loc, kind='stable')
        order[k] = o
        pos_of[k, o] = np.arange(NPC)
        s = np.arange(NPC)
        b, p = s // P, s % P
        trow[k * NPC + o] = k * NPC + p * NB + b
        degs_blk[k] = d_loc[o].reshape(NB, P)

    W = degs_blk.max(axis=(0, 2))              # [NB] shared schedule
    W = np.maximum(W, 1)
    # force a pad at (p=127, last col) for every core (tail-trim guard)
    tailmax = degs_blk[:, :, P - 1].max(axis=0)   # [NB]
    W[tailmax >= W] = tailmax[tailmax >= W] + 1
    CB = np.zeros(NB + 1, np.int64)
    np.cumsum(W, out=CB[1:])
    TC = int(CB[-1])
    assert W.max() <= 44, f"block needs {W.max()} cols > ring-safe 44"

    # greedy whole-block call grouping
    calls = []   # (b0, nblk, cb0, ncols)
    b = 0
    while b < NB:
        b0, cols = b, 0
        while b < NB and cols + W[b] <= MAXCOLS:
            cols += W[b]
            b += 1
        if b == b0:          # oversized single block
            cols = W[b]
            b += 1
        calls.append((b0, b - b0, int(CB[b0]), int(cols)))
    qload = [0, 0, 0, 0]
    qassign = []
    for (_, _, _, ncols) in calls:
        q = qload.index(min(qload))
        qassign.append(q)
        qload[q] += ncols

    idx_all = np.full((NC, TC * P), PADVAL, np.int64)
    s64 = edge_src.astype(np.int64)
    d64 = edge_dst.astype(np.int64)
    kd = d64 // NPC
    for k in range(NC):
        m = kd == k
        dpos = pos_of[k, d64[m] - k * NPC]
        src_r = trow[s64[m]] - N // 2
        o = np.argsort(dpos, kind='stable')
        ds = dpos[o]
        starts = np.r_[0, np.flatnonzero(np.diff(ds)) + 1]
        runlen = np.diff(np.r_[starts, len(ds)])
        cc = np.arange(len(ds)) - np.repeat(starts, runlen)
        bb, pp_ = ds // P, ds % P
        col = CB[bb] + cc
        idx_all[k, col * P + pp_] = src_r[o]
    # wrapped int16 layout [i%16, i//16], replicated to 128 partitions
    idx16 = idx_all.reshape(NC, TC * 8, 16).transpose(0, 2, 1).astype(np.int16)
    idx16 = np.tile(idx16, (1, 8, 1))          # [NC, 128, TC*8]

    negpad = -(W[None, :, None] - degs_blk).astype(np.float32)  # [NC, NB, P]
    negpad = negpad.transpose(0, 2, 1).copy()                   # [NC, P, NB]

    return dict(NPC=NPC, NB=NB, TC=TC, W=W, CB=CB, calls=calls,
                qassign=qassign, idx16=idx16, negpad=negpad, order=order,
                trow=trow, NCHUNK=NCHUNK, BPC=BPC)


def _build(N, pp):
    from concourse import bass, bacc, tile, mybir
    NPC, NB, TC = pp['NPC'], pp['NB'], pp['TC']
    W, CB = pp['W'], pp['CB']
    f32, i16 = mybir.dt.float32, mybir.dt.int16
    AO = mybir.AluOpType
    X = mybir.AxisListType.X

    nc = bacc.Bacc("TRN2", target_bir_lowering=False, debug=False, num_devices=NC,
                   num_swdge_queues=4)
    t_feat = nc.dram_tensor("feat", [P, NB * D], f32, kind="ExternalInput")
    t_tab0 = nc.dram_tensor("tab0", [N, D], f32, kind="ExternalInput")
    t_idx = nc.dram_tensor("idx16", [P, TC * 8], i16, kind="ExternalInput")
    t_norm = nc.dram_tensor("normv", [P, NB], f32, kind="ExternalInput")
    t_norm2 = nc.dram_tensor("norm2v", [P, NB], f32, kind="ExternalInput")
    t_npad = nc.dram_tensor("npadv", [P, NB], f32, kind="ExternalInput")
    t_ident = nc.dram_tensor("ident", [P, P], f32, kind="ExternalInput")
    t_w1T = nc.dram_tensor("w1T", [D, D], f32, kind="ExternalInput")
    t_w2T = nc.dram_tensor("w2T", [D, D], f32, kind="ExternalInput")
    t_b1 = nc.dram_tensor("b1c", [D, 1], f32, kind="ExternalInput")
    t_b2 = nc.dram_tensor("b2b", [P, D], f32, kind="ExternalInput")
    t_gam = nc.dram_tensor("gamb", [P, D], f32, kind="ExternalInput")
    t_bet = nc.dram_tensor("betb", [P, D], f32, kind="ExternalInput")
    t_out = nc.dram_tensor("outp", [P, NB * D], f32, kind="ExternalOutput")
    t_r = nc.dram_tensor("routp", [P, NB * D], f32, kind="ExternalOutput")

    with tile.TileContext(nc) as tc:
        with tc.tile_pool(name="const", bufs=1) as cp, \
             tc.tile_pool(name="work", bufs=1) as wp, \
             tc.tile_pool(name="g", bufs=8) as gp, \
             tc.tile_pool(name="r", bufs=6) as rp, \
             tc.tile_pool(name="oh", bufs=2) as op_, \
             tc.tile_pool(name="ps", bufs=2, space="PSUM") as ps, \
             tc.tile_pool(name="dram", bufs=1, space="DRAM") as dr:

            idx16 = cp.tile([P, TC * 8], i16)
            c3 = pp['calls'][2][2] + pp['calls'][2][3]   # cols thru call 2
            nc.sync.dma_start(out=idx16[:, :c3 * 8], in_=t_idx[:, :c3 * 8])
            nc.sync.dma_start(out=idx16[:, c3 * 8:], in_=t_idx[:, c3 * 8:])
            feat = cp.tile([P, NB * D], f32)
            nrm = cp.tile([P, NB], f32)
            nrm2 = cp.tile([P, NB], f32)
            npad = cp.tile([P, NB], f32)
            idn = cp.tile([P, P], f32)
            w1T = cp.tile([D, D], f32)
            w2T = cp.tile([D, D], f32)
            b1 = cp.tile([D, 1], f32)
            b2b = cp.tile([P, D], f32)
            gmb = cp.tile([P, D], f32)
            btb = cp.tile([P, D], f32)
            for tl, th in ((nrm, t_norm), (nrm2, t_norm2), (npad, t_npad),
                           (feat, t_feat), (idn, t_ident), (w1T, t_w1T),
                           (w2T, t_w2T), (b1, t_b1), (b2b, t_b2),
                           (gmb, t_gam), (btb, t_bet)):
                nc.sync.dma_start(out=tl[:], in_=th[:])

            pub = wp.tile([P, NB * D], f32)
            nh = wp.tile([P, NB * D], f32)
            ag_in = dr.tile([P, NB * D], f32)
            tables = [t_tab0]
            for hh in range(1, HOPS):
                tbl = dr.tile([N, D], f32, addr_space="Shared", tag=f"table{hh}")
                tables.append(tbl)

            def bs(b):
                return slice(b * D, (b + 1) * D)

            GRP = 8
            out_own = wp.tile([P, NB * D], f32)

            def ln_ffn_group(b0, n=GRP):
                """LN + FFN + residuals + store for blocks b0..b0+n-1."""
                sl = slice(b0 * D, (b0 + n) * D)
                r3 = nh[:, sl].rearrange("p (b d) -> p b d", d=D)
                xc = wp.tile([P, GRP * D], f32, tag="xc", bufs=2)
                xc3 = xc[:, :n * D].rearrange("p (b d) -> p b d", d=D)
                sq = wp.tile([P, GRP * D], f32, tag="sq", bufs=2)
                sq3 = sq[:, :n * D].rearrange("p (b d) -> p b d", d=D)
                mu = wp.tile([P, GRP], f32, tag="mu", bufs=2)
                ssq = wp.tile([P, GRP], f32, tag="ssq", bufs=2)
                rstd = wp.tile([P, GRP], f32, tag="rstd", bufs=2)
                nc.vector.tensor_reduce(out=mu[:, :n], in_=r3, axis=X, op=AO.add)
                nc.vector.tensor_scalar(out=mu[:, :n], in0=mu[:, :n], scalar1=1.0 / D,
                                        scalar2=None, op0=AO.mult)
                nc.vector.tensor_tensor(out=xc3, in0=r3,
                                        in1=mu[:, :n].rearrange("p (b o) -> p b o", o=1).to_broadcast([P, n, D]),
                                        op=AO.subtract)
                nc.vector.tensor_tensor(out=sq3, in0=xc3, in1=xc3, op=AO.mult)
                nc.vector.tensor_reduce(out=ssq[:, :n], in_=sq3, axis=X, op=AO.add)
                nc.vector.tensor_scalar(out=ssq[:, :n], in0=ssq[:, :n], scalar1=1.0 / D,
                                        scalar2=None, op0=AO.mult)
                nc.vector.tensor_scalar(out=ssq[:, :n], in0=ssq[:, :n], scalar1=LN_EPS,
                                        scalar2=None, op0=AO.add)
                nc.scalar.activation(out=ssq[:, :n], in_=ssq[:, :n],
                                     func=mybir.ActivationFunctionType.Sqrt)
                nc.vector.reciprocal(rstd[:, :n], ssq[:, :n])
                nc.vector.tensor_tensor(out=xc3, in0=xc3,
                                        in1=rstd[:, :n].rearrange("p (b o) -> p b o", o=1).to_broadcast([P, n, D]),
                                        op=AO.mult)
                nc.vector.tensor_tensor(out=xc3, in0=xc3,
                                        in1=gmb[:].rearrange("p (o d) -> p o d", o=1).to_broadcast([P, n, D]),
                                        op=AO.mult)
                nc.vector.tensor_tensor(out=xc3, in0=xc3,
                                        in1=btb[:].rearrange("p (o d) -> p o d", o=1).to_broadcast([P, n, D]),
                                        op=AO.add)
                for i in range(n):
                    b = b0 + i
                    xT_ps = ps.tile([D, P], f32, tag="tr", space="PSUM")
                    nc.tensor.transpose(out=xT_ps[:], in_=xc[:, i * D:(i + 1) * D],
                                        identity=idn[:])
                    xT = op_.tile([D, P], f32, tag="xT")
                    nc.scalar.copy(xT[:], xT_ps[:])
                    h1_ps = ps.tile([D, P], f32, tag="h1", space="PSUM")
                    nc.tensor.matmul(out=h1_ps[:], lhsT=w1T[:], rhs=xT[:],
                                     start=True, stop=True)
                    h1 = op_.tile([D, P], f32, tag="h1s")
                    nc.scalar.activation(out=h1[:], in_=h1_ps[:],
                                         func=mybir.ActivationFunctionType.Relu,
                                         bias=b1[:, 0:1])
                    ff_ps = ps.tile([P, D], f32, tag="ff", space="PSUM")
                    nc.tensor.matmul(out=ff_ps[:], lhsT=h1[:], rhs=w2T[:],
                                     start=True, stop=True)
                    nc.vector.tensor_tensor(out=out_own[:, bs(b)], in0=ff_ps[:],
                                            in1=nh[:, bs(b)], op=AO.add)
                o3 = out_own[:, sl].rearrange("p (b d) -> p b d", d=D)
                nc.vector.tensor_tensor(out=o3, in0=o3,
                                        in1=feat[:, sl].rearrange("p (b d) -> p b d", d=D),
                                        op=AO.add)
                nc.vector.tensor_tensor(out=o3, in0=o3,
                                        in1=b2b[:].rearrange("p (o d) -> p o d", o=1).to_broadcast([P, n, D]),
                                        op=AO.add)
                nc.sync.dma_start(out=t_out[:, sl], in_=out_own[:, sl])

            rg = [list(range(NC))]
            for hop in range(1, HOPS + 1):
                table = tables[hop - 1]
                if hop > 1:
                    nc.gpsimd.collective_compute("AllGather", AO.bypass,
                                                 replica_groups=rg,
                                                 ins=[ag_in[:]], outs=[table[:]])
                # pad-correction row (table row N-1), replicated to 128 parts
                xu_row = rp.tile([1, D], f32, tag="xur")
                nc.sync.dma_start(out=xu_row[:], in_=table[N - 1:N, :])
                xu = rp.tile([P, D], f32, tag="xu")

                last_hop = hop == HOPS
                for ci, (b0, nblk, cb0, ncols) in enumerate(pp['calls']):
                    g = gp.tile([P, max(MAXCOLS, int(pp["W"].max())), D], f32, tag="g")
                    nc.gpsimd.dma_gather(
                        out_ap=g[:, :ncols, :], in_ap=table[N // 2:, :],
                        idxs_ap=idx16[:, cb0 * 8:(cb0 + ncols) * 8],
                        num_idxs=ncols * P, num_idxs_reg=ncols * P,
                        elem_size=D, single_packet=False,
                        queue_num=pp['qassign'][ci])
                    if ci == 0:
                        nc.gpsimd.partition_broadcast(out_ap=xu[:], in_ap=xu_row[:])
                    for b in range(b0, b0 + nblk):
                        lc = int(CB[b]) - cb0
                        red = rp.tile([P, D], f32, tag="red")
                        nc.vector.tensor_reduce(
                            out=red[:],
                            in_=g[:, lc:lc + int(W[b]), :].rearrange("p c d -> p d c"),
                            axis=X, op=AO.add)
                        corr = rp.tile([P, D], f32, tag="corr")
                        nc.vector.scalar_tensor_tensor(
                            out=corr[:], in0=xu[:], scalar=npad[:, b:b + 1],
                            in1=red[:], op0=AO.mult, op1=AO.add)
                        if not last_hop:
                            nc.scalar.mul(out=pub[:, bs(b)], in_=corr[:],
                                          mul=nrm2[:, b:b + 1])
                            nc.sync.dma_start(out=ag_in[:, bs(b)],
                                              in_=pub[:, bs(b)])
                        else:
                            nc.scalar.mul(out=nh[:, bs(b)], in_=corr[:],
                                          mul=nrm[:, b:b + 1])
                            nc.sync.dma_start(out=t_r[:, bs(b)],
                                              in_=nh[:, bs(b)])
                            if b < NB - 16 and (b + 1) % GRP == 0:
                                ln_ffn_group(b + 1 - GRP)
                            elif b >= NB - 16 and (b + 1) % 4 == 0:
                                ln_ffn_group(b - 3, 4)

    nc.compile()
    return nc


def kernel(features, edge_src, edge_dst, w1, b1, w2, b2, gamma, beta):
    from concourse import bass_utils
    features = np.asarray(features, np.float32)
    edge_src = np.asarray(edge_src, np.int32)
    edge_dst = np.asarray(edge_dst, np.int32)
    N = features.shape[0]
    NPC = N // NC
    NB = NPC // P

    deg = np.bincount(edge_dst, minlength=N).astype(np.float32)
    norm = 1.0 / np.sqrt(np.maximum(deg, 1.0))

    import hashlib
    h = hashlib.sha1()
    h.update(edge_src.tobytes())
    h.update(edge_dst.tobytes())
    h.update(str(N).encode())
    key = h.hexdigest()
    if key not in _CACHE:
        pp = _preprocess(N, edge_src, edge_dst)
        ncb = _build(N, pp)
        _CACHE[key] = (pp, ncb)
    pp, ncb = _CACHE[key]

    # host-computed hop-1 table: norm*features in table-row order
    nf = norm[:, None] * features
    tab0_np = np.empty((N, D), np.float32)
    tab0_np[pp['trow']] = nf

    ident_np = np.eye(P, dtype=np.float32)
    w1T_np = np.ascontiguousarray(np.asarray(w1, np.float32).T)
    w2T_np = np.ascontiguousarray(np.asarray(w2, np.float32).T)
    b1_np = np.asarray(b1, np.float32).reshape(D, 1)
    b2b_np = np.tile(np.asarray(b2, np.float32)[None, :], (P, 1))
    gam_np = np.tile(np.asarray(gamma, np.float32)[None, :], (P, 1))
    bet_np = np.tile(np.asarray(beta, np.float32)[None, :], (P, 1))

    in_maps = []
    for k in range(NC):
        o = pp['order'][k]
        # position s = b*128+p; feat tile [p, b*64+d]
        fo = features[k * NPC + o].reshape(NB, P, D).transpose(1, 0, 2) \
            .reshape(P, NB * D).copy()
        no = norm[k * NPC + o].reshape(NB, P).T.copy()
        in_maps.append({
            "feat": fo, "tab0": tab0_np, "idx16": pp['idx16'][k],
            "normv": no, "norm2v": (no * no), "npadv": pp['negpad'][k],
            "ident": ident_np,
            "w1T": w1T_np, "w2T": w2T_np, "b1c": b1_np, "b2b": b2b_np,
            "gamb": gam_np, "betb": bet_np,
        })

    trace = os.environ.get("GCN_TRACE", "0") == "1"
    res = bass_utils.run_bass_kernel_spmd(ncb, in_maps, core_ids=list(range(NC)),
                                          trace=trace)
    if trace and res.exec_time_ns is not None:
        print(f"HW exec time: {res.exec_time_ns} ns")
    if trace and res.instructions_and_trace is not None:
        print(f"Trace path: {res.instructions_and_trace[1]}")

    out = np.empty((N, D), np.float32)
    r = np.empty((N, D), np.float32)
    for k in range(NC):
        o = res.results[k]["outp"].reshape(P, NB, D).transpose(1, 0, 2).reshape(NPC, D)
        rr = res.results[k]["routp"].reshape(P, NB, D).transpose(1, 0, 2).reshape(NPC, D)
        out[k * NPC + pp['order'][k]] = o
        r[k * NPC + pp['order'][k]] = rr
    return (out, r)


# revision 22
# speedup vs baseline: 1.0235x; 1.0235x over previous
"""GCN block (3-hop symmetric-normalized propagation + LN/FFN/residual) on 8 trn2 cores.

v4: identity-slot gather (no one-hots, no scatter matmuls).
  - Table per hop: [65536, 64] fp32 (256B rows) in DRAM. Signed int16 gather
    indices with the base biased to row 32768 address all 65536 rows
    (idx = row - 32768; the Q7 address math sign-extends correctly).
  - Nodes sharded 8 ways; per core, dst nodes are degree-sorted (desc) into
    64 blocks of 128. Block b gets W[b] = max in-block degree columns
    (shared schedule = max over cores). Edge e of dst (b,p) lands at
    gather slot [partition p, column CB[b]+j] -- directly in destination
    position. Pad slots point at row 65535 (idx +32767, also guarantees the
    per-call idx tail is non-negative so the uCode trim never fires).
  - Aggregation per block = ONE strided tensor_reduce over the block's
    columns (DVE), then a rank-1 pad correction (scalar_tensor_tensor with
    per-partition pad counts x table row 65535) and the norm scale on the
    scalar engine. GPSIMD does nothing but desc-gen, on 4 SWDGE queues.
  - Hop-1 table = norm*features, host-computed. Hops 2/3 tables via
    AllGather of each core's scaled block results (fp32, 2MB/core).
  - LN + FFN fp32, interleaved per 8-block group under the hop-3 stream.
"""
import sys
sys.path.insert(0, '/opt/trn_rl_repo')
import os
import numpy as np

NC = 8          # cores
P = 128         # partitions
D = 64          # feature dim
HOPS = 3
LN_EPS = 1e-5
MAXCOLS = 24    # max gather columns per call (ring-safe; small calls pipeline better)
PADVAL = 32767  # pad idx -> table row 65535

_CACHE = {}


def _preprocess(N, edge_src, edge_dst):
    NPC = N // NC          # nodes per core (8192)
    NB = NPC // P          # dst blocks per core (64)

    deg = np.bincount(edge_dst, minlength=N).astype(np.int64)

    NCHUNK = 4                                 # AllGather chunks per hop
    BPC = NB // NCHUNK                         # blocks per chunk (16)
    order = np.empty((NC, NPC), np.int64)      # position s -> local node
    pos_of = np.empty((NC, NPC), np.int64)     # local node -> position s
    trow = np.empty(N, np.int64)               # node -> table row
    degs_blk = np.empty((NC, NB, P), np.int64)
    for k in range(NC):
        d_loc = deg[k * NPC:(k + 1) * NPC]
        o = np.argsort(-d_loc, kind='stable')
        order[k] = o
        pos_of[k, o] = np.arange(NPC)
        s = np.arange(NPC)
        b, p = s // P, s % P
        trow[k * NPC + o] = k * NPC + p * NB + b
        degs_blk[k] = d_loc[o].reshape(NB, P)

    W = degs_blk.max(axis=(0, 2))              # [NB] shared schedule
    W = np.maximum(W, 1)
    # force a pad at (p=127, last col) for every core (tail-trim guard)
    tailmax = degs_blk[:, :, P - 1].max(axis=0)   # [NB]
    W[tailmax >= W] = tailmax[tailmax >= W] + 1
    CB = np.zeros(NB + 1, np.int64)
    np.cumsum(W, out=CB[1:])
    TC = int(CB[-1])
    assert W.max() <= 44, f"block needs {W.max()} cols > ring-safe 44"

    # greedy whole-block call grouping
    calls = []   # (b0, nblk, cb0, ncols)
    b = 0
    while b < NB:
        b0, cols = b, 0
        while b < NB and cols + W[b] <= MAXCOLS:
            cols += W[b]
            b += 1
        if b == b0:          # oversized single block
            cols = W[b]
            b += 1
        calls.append((b0, b - b0, int(CB[b0]), int(cols)))
    qload = [0, 0, 0, 0]
    qassign = []
    for (_, _, _, ncols) in calls:
        q = qload.index(min(qload))
        qassign.append(q)
        qload[q] += ncols

    idx_all = np.full((NC, TC * P), PADVAL, np.int64)
    s64 = edge_src.astype(np.int64)
    d64 = edge_dst.astype(np.int64)
    kd = d64 // NPC
    for k in range(NC):
        m = kd == k
        dpos = pos_of[k, d64[m] - k * NPC]
        src_r = trow[s64[m]] - N // 2
        o = np.argsort(dpos, kind='stable')
        ds = dpos[o]
        starts = np.r_[0, np.flatnonzero(np.diff(ds)) + 1]
        runlen = np.diff(np.r_[starts, len(ds)])
        cc = np.arange(len(ds)) - np.repeat(starts, runlen)
        bb, pp_ = ds // P, ds % P
        col = CB[bb] + cc
        idx_all[k, col * P + pp_] = src_r[o]
    # wrapped int16 layout [i%16, i//16], replicated to 128 partitions
    idx16 = idx_all.reshape(NC, TC * 8, 16).transpose(0, 2, 1).astype(np.int16)
    idx16 = np.tile(idx16, (1, 8, 1))          # [NC, 128, TC*8]

    negpad = -(W[None, :, None] - degs_blk).astype(np.float32)  # [NC, NB, P]
    negpad = negpad.transpose(0, 2, 1).copy()                   # [NC, P, NB]

    return dict(NPC=NPC, NB=NB, TC=TC, W=W, CB=CB, calls=calls,
                qassign=qassign, idx16=idx16, negpad=negpad, order=order,
                trow=trow, NCHUNK=NCHUNK, BPC=BPC)


def _build(N, pp):
    from concourse import bass, bacc, tile, mybir
    NPC, NB, TC = pp['NPC'], pp['NB'], pp['TC']
    W, CB = pp['W'], pp['CB']
    f32, i16 = mybir.dt.float32, mybir.dt.int16
    AO = mybir.AluOpType
    X = mybir.AxisListType.X

    nc = bacc.Bacc("TRN2", target_bir_lowering=False, debug=False, num_devices=NC,
                   num_swdge_queues=4)
    t_feat = nc.dram_tensor("feat", [P, NB * D], f32, kind="ExternalInput")
    t_tab0 = nc.dram_tensor("tab0", [N, D], f32, kind="ExternalInput")
    t_idx = nc.dram_tensor("idx16", [P, TC * 8], i16, kind="ExternalInput")
    t_norm = nc.dram_tensor("normv", [P, NB], f32, kind="ExternalInput")
    t_norm2 = nc.dram_tensor("norm2v", [P, NB], f32, kind="ExternalInput")
    t_npad = nc.dram_tensor("npadv", [P, NB], f32, kind="ExternalInput")
    t_ident = nc.dram_tensor("ident", [P, P], f32, kind="ExternalInput")
    t_w1T = nc.dram_tensor("w1T", [D, D], f32, kind="ExternalInput")
    t_w2T = nc.dram_tensor("w2T", [D, D], f32, kind="ExternalInput")
    t_b1 = nc.dram_tensor("b1c", [D, 1], f32, kind="ExternalInput")
    t_b2 = nc.dram_tensor("b2b", [P, D], f32, kind="ExternalInput")
    t_gam = nc.dram_tensor("gamb", [P, D], f32, kind="ExternalInput")
    t_bet = nc.dram_tensor("betb", [P, D], f32, kind="ExternalInput")
    t_out = nc.dram_tensor("outp", [P, NB * D], f32, kind="ExternalOutput")
    t_r = nc.dram_tensor("routp", [P, NB * D], f32, kind="ExternalOutput")

    with tile.TileContext(nc) as tc:
        with tc.tile_pool(name="const", bufs=1) as cp, \
             tc.tile_pool(name="work", bufs=1) as wp, \
             tc.tile_pool(name="g", bufs=12) as gp, \
             tc.tile_pool(name="r", bufs=6) as rp, \
             tc.tile_pool(name="oh", bufs=2) as op_, \
             tc.tile_pool(name="ps", bufs=2, space="PSUM") as ps, \
             tc.tile_pool(name="dram", bufs=1, space="DRAM") as dr:

            idx16 = cp.tile([P, TC * 8], i16)
            c3 = pp['calls'][2][2] + pp['calls'][2][3]   # cols thru call 2
            nc.sync.dma_start(out=idx16[:, :c3 * 8], in_=t_idx[:, :c3 * 8])
            nc.sync.dma_start(out=idx16[:, c3 * 8:], in_=t_idx[:, c3 * 8:])
            feat = cp.tile([P, NB * D], f32)
            nrm = cp.tile([P, NB], f32)
            nrm2 = cp.tile([P, NB], f32)
            npad = cp.tile([P, NB], f32)
            idn = cp.tile([P, P], f32)
            w1T = cp.tile([D, D], f32)
            w2T = cp.tile([D, D], f32)
            b1 = cp.tile([D, 1], f32)
            b2b = cp.tile([P, D], f32)
            gmb = cp.tile([P, D], f32)
            btb = cp.tile([P, D], f32)
            for tl, th in ((nrm, t_norm), (nrm2, t_norm2), (npad, t_npad),
                           (feat, t_feat), (idn, t_ident), (w1T, t_w1T),
                           (w2T, t_w2T), (b1, t_b1), (b2b, t_b2),
                           (gmb, t_gam), (btb, t_bet)):
                nc.sync.dma_start(out=tl[:], in_=th[:])

            nh = wp.tile([P, NB * D], f32)
            ag_in = dr.tile([P, NB * D], f32)
            tables = [t_tab0]
            for hh in range(1, HOPS):
                tbl = dr.tile([N, D], f32, addr_space="Shared", tag=f"table{hh}")
                tables.append(tbl)

            def bs(b):
                return slice(b * D, (b + 1) * D)

            GRP = 8
            out_own = wp.tile([P, NB * D], f32)

            def ln_ffn_group(b0, n=GRP):
                """LN + FFN + residuals + store for blocks b0..b0+n-1."""
                sl = slice(b0 * D, (b0 + n) * D)
                r3 = nh[:, sl].rearrange("p (b d) -> p b d", d=D)
                xc = wp.tile([P, GRP * D], f32, tag="xc", bufs=2)
                xc3 = xc[:, :n * D].rearrange("p (b d) -> p b d", d=D)
                sq = wp.tile([P, GRP * D], f32, tag="sq", bufs=2)
                sq3 = sq[:, :n * D].rearrange("p (b d) -> p b d", d=D)
                mu = wp.tile([P, GRP], f32, tag="mu", bufs=2)
                ssq = wp.tile([P, GRP], f32, tag="ssq", bufs=2)
                rstd = wp.tile([P, GRP], f32, tag="rstd", bufs=2)
                nc.vector.tensor_reduce(out=mu[:, :n], in_=r3, axis=X, op=AO.add)
                nc.vector.tensor_scalar(out=mu[:, :n], in0=mu[:, :n], scalar1=1.0 / D,
                                        scalar2=None, op0=AO.mult)
                nc.vector.tensor_tensor(out=xc3, in0=r3,
                                        in1=mu[:, :n].rearrange("p (b o) -> p b o", o=1).to_broadcast([P, n, D]),
                                        op=AO.subtract)
                nc.vector.tensor_tensor(out=sq3, in0=xc3, in1=xc3, op=AO.mult)
                nc.vector.tensor_reduce(out=ssq[:, :n], in_=sq3, axis=X, op=AO.add)
                nc.vector.tensor_scalar(out=ssq[:, :n], in0=ssq[:, :n], scalar1=1.0 / D,
                                        scalar2=None, op0=AO.mult)
                nc.vector.tensor_scalar(out=ssq[:, :n], in0=ssq[:, :n], scalar1=LN_EPS,
                                        scalar2=None, op0=AO.add)
                nc.scalar.activation(out=ssq[:, :n], in_=ssq[:, :n],
                                     func=mybir.ActivationFunctionType.Sqrt)
                nc.vector.reciprocal(rstd[:, :n], ssq[:, :n])
                nc.vector.tensor_tensor(out=xc3, in0=xc3,
                                        in1=rstd[:, :n].rearrange("p (b o) -> p b o", o=1).to_broadcast([P, n, D]),
                                        op=AO.mult)
                nc.vector.tensor_tensor(out=xc3, in0=xc3,
                                        in1=gmb[:].rearrange("p (o d) -> p o d", o=1).to_broadcast([P, n, D]),
                                        op=AO.mult)
                nc.vector.tensor_tensor(out=xc3, in0=xc3,
                                        in1=btb[:].rearrange("p (o d) -> p o d", o=1).to_broadcast([P, n, D]),
                                        op=AO.add)
                for i in range(n):
                    b = b0 + i
                    xT_ps = ps.tile([D, P], f32, tag="tr", space="PSUM")
                    nc.tensor.transpose(out=xT_ps[:], in_=xc[:, i * D:(i + 1) * D],
                                        identity=idn[:])
                    xT = op_.tile([D, P], f32, tag="xT")
                    nc.scalar.copy(xT[:], xT_ps[:])
                    h1_ps = ps.tile([D, P], f32, tag="h1", space="PSUM")
                    nc.tensor.matmul(out=h1_ps[:], lhsT=w1T[:], rhs=xT[:],
                                     start=True, stop=True)
                    h1 = op_.tile([D, P], f32, tag="h1s")
                    nc.scalar.activation(out=h1[:], in_=h1_ps[:],
                                         func=mybir.ActivationFunctionType.Relu,
                                         bias=b1[:, 0:1])
                    ff_ps = ps.tile([P, D], f32, tag="ff", space="PSUM")
                    nc.tensor.matmul(out=ff_ps[:], lhsT=h1[:], rhs=w2T[:],
                                     start=True, stop=True)
                    nc.vector.tensor_tensor(out=out_own[:, bs(b)], in0=ff_ps[:],
                                            in1=nh[:, bs(b)], op=AO.add)
                o3 = out_own[:, sl].rearrange("p (b d) -> p b d", d=D)
                nc.vector.tensor_tensor(out=o3, in0=o3,
                                        in1=feat[:, sl].rearrange("p (b d) -> p b d", d=D),
                                        op=AO.add)
                nc.vector.tensor_tensor(out=o3, in0=o3,
                                        in1=b2b[:].rearrange("p (o d) -> p o d", o=1).to_broadcast([P, n, D]),
                                        op=AO.add)
                nc.sync.dma_start(out=t_out[:, sl], in_=out_own[:, sl])

            rg = [list(range(NC))]
            for hop in range(1, HOPS + 1):
                table = tables[hop - 1]
                if hop > 1:
                    nc.gpsimd.collective_compute("AllGather", AO.bypass,
                                                 replica_groups=rg,
                                                 ins=[ag_in[:]], outs=[table[:]])
                # pad-correction row (table row N-1), replicated to 128 parts
                xu_row = rp.tile([1, D], f32, tag="xur")
                nc.sync.dma_start(out=xu_row[:], in_=table[N - 1:N, :])
                xu = rp.tile([P, D], f32, tag="xu")

                last_hop = hop == HOPS
                for ci, (b0, nblk, cb0, ncols) in enumerate(pp['calls']):
                    g = gp.tile([P, max(MAXCOLS, int(pp["W"].max())), D], f32, tag="g")
                    nc.gpsimd.dma_gather(
                        out_ap=g[:, :ncols, :], in_ap=table[N // 2:, :],
                        idxs_ap=idx16[:, cb0 * 8:(cb0 + ncols) * 8],
                        num_idxs=ncols * P, num_idxs_reg=ncols * P,
                        elem_size=D, single_packet=False,
                        queue_num=pp['qassign'][ci])
                    if ci == 0:
                        nc.gpsimd.partition_broadcast(out_ap=xu[:], in_ap=xu_row[:])
                    for b in range(b0, b0 + nblk):
                        lc = int(CB[b]) - cb0
                        red = rp.tile([P, D], f32, tag="red")
                        nc.vector.tensor_reduce(
                            out=red[:],
                            in_=g[:, lc:lc + int(W[b]), :].rearrange("p c d -> p d c"),
                            axis=X, op=AO.add)
                        corr = rp.tile([P, D], f32, tag="corr")
                        nc.vector.scalar_tensor_tensor(
                            out=corr[:], in0=xu[:], scalar=npad[:, b:b + 1],
                            in1=red[:], op0=AO.mult, op1=AO.add)
                        if not last_hop:
                            nc.scalar.mul(out=nh[:, bs(b)], in_=corr[:],
                                          mul=nrm2[:, b:b + 1])
                            nc.sync.dma_start(out=ag_in[:, bs(b)],
                                              in_=nh[:, bs(b)])
                        else:
                            nc.scalar.mul(out=nh[:, bs(b)], in_=corr[:],
                                          mul=nrm[:, b:b + 1])
                            nc.sync.dma_start(out=t_r[:, bs(b)],
                                              in_=nh[:, bs(b)])
                            if b < NB - 16 and (b + 1) % GRP == 0:
                                ln_ffn_group(b + 1 - GRP)
                            elif b >= NB - 16 and (b + 1) % 4 == 0:
                                ln_ffn_group(b - 3, 4)

    nc.compile()
    return nc


def kernel(features, edge_src, edge_dst, w1, b1, w2, b2, gamma, beta):
    from concourse import bass_utils
    features = np.asarray(features, np.float32)
    edge_src = np.asarray(edge_src, np.int32)
    edge_dst = np.asarray(edge_dst, np.int32)
    N = features.shape[0]
    NPC = N // NC
    NB = NPC // P

    deg = np.bincount(edge_dst, minlength=N).astype(np.float32)
    norm = 1.0 / np.sqrt(np.maximum(deg, 1.0))

    import hashlib
    h = hashlib.sha1()
    h.update(edge_src.tobytes())
    h.update(edge_dst.tobytes())
    h.update(str(N).encode())
    key = h.hexdigest()
    if key not in _CACHE:
        pp = _preprocess(N, edge_src, edge_dst)
        ncb = _build(N, pp)
        _CACHE[key] = (pp, ncb)
    pp, ncb = _CACHE[key]

    # host-computed hop-1 table: norm*features in table-row order
    nf = norm[:, None] * features
    tab0_np = np.empty((N, D), np.float32)
    tab0_np[pp['trow']] = nf

    ident_np = np.eye(P, dtype=np.float32)
    w1T_np = np.ascontiguousarray(np.asarray(w1, np.float32).T)
    w2T_np = np.ascontiguousarray(np.asarray(w2, np.float32).T)
    b1_np = np.asarray(b1, np.float32).reshape(D, 1)
    b2b_np = np.tile(np.asarray(b2, np.float32)[None, :], (P, 1))
    gam_np = np.tile(np.asarray(gamma, np.float32)[None, :], (P, 1))
    bet_np = np.tile(np.asarray(beta, np.float32)[None, :], (P, 1))

    in_maps = []
    for k in range(NC):
        o = pp['order'][k]
        # position s = b*128+p; feat tile [p, b*64+d]
        fo = features[k * NPC + o].reshape(NB, P, D).transpose(1, 0, 2) \
            .reshape(P, NB * D).copy()
        no = norm[k * NPC + o].reshape(NB, P).T.copy()
        in_maps.append({
            "feat": fo, "tab0": tab0_np, "idx16": pp['idx16'][k],
            "normv": no, "norm2v": (no * no), "npadv": pp['negpad'][k],
            "ident": ident_np,
            "w1T": w1T_np, "w2T": w2T_np, "b1c": b1_np, "b2b": b2b_np,
            "gamb": gam_np, "betb": bet_np,
        })

    trace = os.environ.get("GCN_TRACE", "0") == "1"
    res = bass_utils.run_bass_kernel_spmd(ncb, in_maps, core_ids=list(range(NC)),
                                          trace=trace)
    if trace and res.exec_time_ns is not None:
        print(f"HW exec time: {res.exec_time_ns} ns")
    if trace and res.instructions_and_trace is not None:
        print(f"Trace path: {res.instructions_and_trace[1]}")

    out = np.empty((N, D), np.float32)
    r = np.empty((N, D), np.float32)
    for k in range(NC):
        o = res.results[k]["outp"].reshape(P, NB, D).transpose(1, 0, 2).reshape(NPC, D)
        rr = res.results[k]["routp"].reshape(P, NB, D).transpose(1, 0, 2).reshape(NPC, D)
        out[k * NPC + pp['order'][k]] = o
        r[k * NPC + pp['order'][k]] = rr
    return (out, r)


# revision 23
# speedup vs baseline: 1.0246x; 1.0010x over previous
"""GCN block (3-hop symmetric-normalized propagation + LN/FFN/residual) on 8 trn2 cores.

v4: identity-slot gather (no one-hots, no scatter matmuls).
  - Table per hop: [65536, 64] fp32 (256B rows) in DRAM. Signed int16 gather
    indices with the base biased to row 32768 address all 65536 rows
    (idx = row - 32768; the Q7 address math sign-extends correctly).
  - Nodes sharded 8 ways; per core, dst nodes are degree-sorted (desc) into
    64 blocks of 128. Block b gets W[b] = max in-block degree columns
    (shared schedule = max over cores). Edge e of dst (b,p) lands at
    gather slot [partition p, column CB[b]+j] -- directly in destination
    position. Pad slots point at row 65535 (idx +32767, also guarantees the
    per-call idx tail is non-negative so the uCode trim never fires).
  - Aggregation per block = ONE strided tensor_reduce over the block's
    columns (DVE), then a rank-1 pad correction (scalar_tensor_tensor with
    per-partition pad counts x table row 65535) and the norm scale on the
    scalar engine. GPSIMD does nothing but desc-gen, on 4 SWDGE queues.
  - Hop-1 table = norm*features, host-computed. Hops 2/3 tables via
    AllGather of each core's scaled block results (fp32, 2MB/core).
  - LN + FFN fp32, interleaved per 8-block group under the hop-3 stream.
"""
import sys
sys.path.insert(0, '/opt/trn_rl_repo')
import os
import numpy as np

NC = 8          # cores
P = 128         # partitions
D = 64          # feature dim
HOPS = 3
LN_EPS = 1e-5
MAXCOLS = 24    # max gather columns per call (ring-safe; small calls pipeline better)
PADVAL = 32767  # pad idx -> table row 65535

_CACHE = {}


def _preprocess(N, edge_src, edge_dst):
    NPC = N // NC          # nodes per core (8192)
    NB = NPC // P          # dst blocks per core (64)

    deg = np.bincount(edge_dst, minlength=N).astype(np.int64)

    NCHUNK = 4                                 # AllGather chunks per hop
    BPC = NB // NCHUNK                         # blocks per chunk (16)
    order = np.empty((NC, NPC), np.int64)      # position s -> local node
    pos_of = np.empty((NC, NPC), np.int64)     # local node -> position s
    trow = np.empty(N, np.int64)               # node -> table row
    degs_blk = np.empty((NC, NB, P), np.int64)
    for k in range(NC):
        d_loc = deg[k * NPC:(k + 1) * NPC]
        o = np.argsort(-d_loc, kind='stable')
        order[k] = o
        pos_of[k, o] = np.arange(NPC)
        s = np.arange(NPC)
        b, p = s // P, s % P
        trow[k * NPC + o] = k * NPC + p * NB + b
        degs_blk[k] = d_loc[o].reshape(NB, P)

    W = degs_blk.max(axis=(0, 2))              # [NB] shared schedule
    W = np.maximum(W, 1)
    # force a pad at (p=127, last col) for every core (tail-trim guard)
    tailmax = degs_blk[:, :, P - 1].max(axis=0)   # [NB]
    W[tailmax >= W] = tailmax[tailmax >= W] + 1
    CB = np.zeros(NB + 1, np.int64)
    np.cumsum(W, out=CB[1:])
    TC = int(CB[-1])
    assert W.max() <= 44, f"block needs {W.max()} cols > ring-safe 44"

    # greedy whole-block call grouping
    calls = []   # (b0, nblk, cb0, ncols)
    b = 0
    while b < NB:
        b0, cols = b, 0
        while b < NB and cols + W[b] <= MAXCOLS:
            cols += W[b]
            b += 1
        if b == b0:          # oversized single block
            cols = W[b]
            b += 1
        calls.append((b0, b - b0, int(CB[b0]), int(cols)))
    qload = [0, 0, 0, 0]
    qassign = []
    for (_, _, _, ncols) in calls:
        q = qload.index(min(qload))
        qassign.append(q)
        qload[q] += ncols

    idx_all = np.full((NC, TC * P), PADVAL, np.int64)
    s64 = edge_src.astype(np.int64)
    d64 = edge_dst.astype(np.int64)
    kd = d64 // NPC
    for k in range(NC):
        m = kd == k
        dpos = pos_of[k, d64[m] - k * NPC]
        src_r = trow[s64[m]] - N // 2
        # sort by (dst, ascending src row): column c of a block then holds
        # similar-quantile rows -> better HBM locality for the random reads
        o = np.lexsort((src_r, dpos))
        ds = dpos[o]
        starts = np.r_[0, np.flatnonzero(np.diff(ds)) + 1]
        runlen = np.diff(np.r_[starts, len(ds)])
        cc = np.arange(len(ds)) - np.repeat(starts, runlen)
        bb, pp_ = ds // P, ds % P
        col = CB[bb] + cc
        idx_all[k, col * P + pp_] = src_r[o]
    # wrapped int16 layout [i%16, i//16], replicated to 128 partitions
    idx16 = idx_all.reshape(NC, TC * 8, 16).transpose(0, 2, 1).astype(np.int16)
    idx16 = np.tile(idx16, (1, 8, 1))          # [NC, 128, TC*8]

    negpad = -(W[None, :, None] - degs_blk).astype(np.float32)  # [NC, NB, P]
    negpad = negpad.transpose(0, 2, 1).copy()                   # [NC, P, NB]

    return dict(NPC=NPC, NB=NB, TC=TC, W=W, CB=CB, calls=calls,
                qassign=qassign, idx16=idx16, negpad=negpad, order=order,
                trow=trow, NCHUNK=NCHUNK, BPC=BPC)


def _build(N, pp):
    from concourse import bass, bacc, tile, mybir
    NPC, NB, TC = pp['NPC'], pp['NB'], pp['TC']
    W, CB = pp['W'], pp['CB']
    f32, i16 = mybir.dt.float32, mybir.dt.int16
    AO = mybir.AluOpType
    X = mybir.AxisListType.X

    nc = bacc.Bacc("TRN2", target_bir_lowering=False, debug=False, num_devices=NC,
                   num_swdge_queues=4)
    t_feat = nc.dram_tensor("feat", [P, NB * D], f32, kind="ExternalInput")
    t_tab0 = nc.dram_tensor("tab0", [N, D], f32, kind="ExternalInput")
    t_idx = nc.dram_tensor("idx16", [P, TC * 8], i16, kind="ExternalInput")
    t_norm = nc.dram_tensor("normv", [P, NB], f32, kind="ExternalInput")
    t_norm2 = nc.dram_tensor("norm2v", [P, NB], f32, kind="ExternalInput")
    t_npad = nc.dram_tensor("npadv", [P, NB], f32, kind="ExternalInput")
    t_ident = nc.dram_tensor("ident", [P, P], f32, kind="ExternalInput")
    t_w1T = nc.dram_tensor("w1T", [D, D], f32, kind="ExternalInput")
    t_w2T = nc.dram_tensor("w2T", [D, D], f32, kind="ExternalInput")
    t_b1 = nc.dram_tensor("b1c", [D, 1], f32, kind="ExternalInput")
    t_b2 = nc.dram_tensor("b2b", [P, D], f32, kind="ExternalInput")
    t_gam = nc.dram_tensor("gamb", [P, D], f32, kind="ExternalInput")
    t_bet = nc.dram_tensor("betb", [P, D], f32, kind="ExternalInput")
    t_out = nc.dram_tensor("outp", [P, NB * D], f32, kind="ExternalOutput")
    t_r = nc.dram_tensor("routp", [P, NB * D], f32, kind="ExternalOutput")

    with tile.TileContext(nc) as tc:
        with tc.tile_pool(name="const", bufs=1) as cp, \
             tc.tile_pool(name="work", bufs=1) as wp, \
             tc.tile_pool(name="g", bufs=12) as gp, \
             tc.tile_pool(name="r", bufs=6) as rp, \
             tc.tile_pool(name="oh", bufs=2) as op_, \
             tc.tile_pool(name="ps", bufs=2, space="PSUM") as ps, \
             tc.tile_pool(name="dram", bufs=1, space="DRAM") as dr:

            idx16 = cp.tile([P, TC * 8], i16)
            c3 = pp['calls'][2][2] + pp['calls'][2][3]   # cols thru call 2
            nc.sync.dma_start(out=idx16[:, :c3 * 8], in_=t_idx[:, :c3 * 8])
            nc.sync.dma_start(out=idx16[:, c3 * 8:], in_=t_idx[:, c3 * 8:])
            feat = cp.tile([P, NB * D], f32)
            nrm = cp.tile([P, NB], f32)
            nrm2 = cp.tile([P, NB], f32)
            npad = cp.tile([P, NB], f32)
            idn = cp.tile([P, P], f32)
            w1T = cp.tile([D, D], f32)
            w2T = cp.tile([D, D], f32)
            b1 = cp.tile([D, 1], f32)
            b2b = cp.tile([P, D], f32)
            gmb = cp.tile([P, D], f32)
            btb = cp.tile([P, D], f32)
            for tl, th in ((nrm, t_norm), (nrm2, t_norm2), (npad, t_npad),
                           (feat, t_feat), (idn, t_ident), (w1T, t_w1T),
                           (w2T, t_w2T), (b1, t_b1), (b2b, t_b2),
                           (gmb, t_gam), (btb, t_bet)):
                nc.sync.dma_start(out=tl[:], in_=th[:])

            nh = wp.tile([P, NB * D], f32)
            ag_in = dr.tile([P, NB * D], f32)
            tables = [t_tab0]
            for hh in range(1, HOPS):
                tbl = dr.tile([N, D], f32, addr_space="Shared", tag=f"table{hh}")
                tables.append(tbl)

            def bs(b):
                return slice(b * D, (b + 1) * D)

            GRP = 8
            out_own = wp.tile([P, NB * D], f32)

            def ln_ffn_group(b0, n=GRP):
                """LN + FFN + residuals + store for blocks b0..b0+n-1."""
                sl = slice(b0 * D, (b0 + n) * D)
                r3 = nh[:, sl].rearrange("p (b d) -> p b d", d=D)
                xc = wp.tile([P, GRP * D], f32, tag="xc", bufs=2)
                xc3 = xc[:, :n * D].rearrange("p (b d) -> p b d", d=D)
                sq = wp.tile([P, GRP * D], f32, tag="sq", bufs=2)
                sq3 = sq[:, :n * D].rearrange("p (b d) -> p b d", d=D)
                mu = wp.tile([P, GRP], f32, tag="mu", bufs=2)
                ssq = wp.tile([P, GRP], f32, tag="ssq", bufs=2)
                rstd = wp.tile([P, GRP], f32, tag="rstd", bufs=2)
                nc.vector.tensor_reduce(out=mu[:, :n], in_=r3, axis=X, op=AO.add)
                nc.vector.tensor_scalar(out=mu[:, :n], in0=mu[:, :n], scalar1=1.0 / D,
                                        scalar2=None, op0=AO.mult)
                nc.vector.tensor_tensor(out=xc3, in0=r3,
                                        in1=mu[:, :n].rearrange("p (b o) -> p b o", o=1).to_broadcast([P, n, D]),
                                        op=AO.subtract)
                nc.vector.tensor_tensor(out=sq3, in0=xc3, in1=xc3, op=AO.mult)
                nc.vector.tensor_reduce(out=ssq[:, :n], in_=sq3, axis=X, op=AO.add)
                nc.vector.tensor_scalar(out=ssq[:, :n], in0=ssq[:, :n], scalar1=1.0 / D,
                                        scalar2=None, op0=AO.mult)
                nc.vector.tensor_scalar(out=ssq[:, :n], in0=ssq[:, :n], scalar1=LN_EPS,
                                        scalar2=None, op0=AO.add)
                nc.scalar.activation(out=ssq[:, :n], in_=ssq[:, :n],
                                     func=mybir.ActivationFunctionType.Sqrt)
                nc.vector.reciprocal(rstd[:, :n], ssq[:, :n])
                nc.vector.tensor_tensor(out=xc3, in0=xc3,
                                        in1=rstd[:, :n].rearrange("p (b o) -> p b o", o=1).to_broadcast([P, n, D]),
                                        op=AO.mult)
                nc.vector.tensor_tensor(out=xc3, in0=xc3,
                                        in1=gmb[:].rearrange("p (o d) -> p o d", o=1).to_broadcast([P, n, D]),
                                        op=AO.mult)
                nc.vector.tensor_tensor(out=xc3, in0=xc3,
                                        in1=btb[:].rearrange("p (o d) -> p o d", o=1).to_broadcast([P, n, D]),
                                        op=AO.add)
                for i in range(n):
                    b = b0 + i
                    xT_ps = ps.tile([D, P], f32, tag="tr", space="PSUM")
                    nc.tensor.transpose(out=xT_ps[:], in_=xc[:, i * D:(i + 1) * D],
                                        identity=idn[:])
                    xT = op_.tile([D, P], f32, tag="xT")
                    nc.scalar.copy(xT[:], xT_ps[:])
                    h1_ps = ps.tile([D, P], f32, tag="h1", space="PSUM")
                    nc.tensor.matmul(out=h1_ps[:], lhsT=w1T[:], rhs=xT[:],
                                     start=True, stop=True)
                    h1 = op_.tile([D, P], f32, tag="h1s")
                    nc.scalar.activation(out=h1[:], in_=h1_ps[:],
                                         func=mybir.ActivationFunctionType.Relu,
                                         bias=b1[:, 0:1])
                    ff_ps = ps.tile([P, D], f32, tag="ff", space="PSUM")
                    nc.tensor.matmul(out=ff_ps[:], lhsT=h1[:], rhs=w2T[:],
                                     start=True, stop=True)
                    nc.vector.tensor_tensor(out=out_own[:, bs(b)], in0=ff_ps[:],
                                            in1=nh[:, bs(b)], op=AO.add)
                o3 = out_own[:, sl].rearrange("p (b d) -> p b d", d=D)
                nc.vector.tensor_tensor(out=o3, in0=o3,
                                        in1=feat[:, sl].rearrange("p (b d) -> p b d", d=D),
                                        op=AO.add)
                nc.vector.tensor_tensor(out=o3, in0=o3,
                                        in1=b2b[:].rearrange("p (o d) -> p o d", o=1).to_broadcast([P, n, D]),
                                        op=AO.add)
                nc.sync.dma_start(out=t_out[:, sl], in_=out_own[:, sl])

            rg = [list(range(NC))]
            for hop in range(1, HOPS + 1):
                table = tables[hop - 1]
                if hop > 1:
                    nc.gpsimd.collective_compute("AllGather", AO.bypass,
                                                 replica_groups=rg,
                                                 ins=[ag_in[:]], outs=[table[:]])
                # pad-correction row (table row N-1), replicated to 128 parts
                xu_row = rp.tile([1, D], f32, tag="xur")
                nc.sync.dma_start(out=xu_row[:], in_=table[N - 1:N, :])
                xu = rp.tile([P, D], f32, tag="xu")

                last_hop = hop == HOPS
                for ci, (b0, nblk, cb0, ncols) in enumerate(pp['calls']):
                    g = gp.tile([P, max(MAXCOLS, int(pp["W"].max())), D], f32, tag="g")
                    nc.gpsimd.dma_gather(
                        out_ap=g[:, :ncols, :], in_ap=table[N // 2:, :],
                        idxs_ap=idx16[:, cb0 * 8:(cb0 + ncols) * 8],
                        num_idxs=ncols * P, num_idxs_reg=ncols * P,
                        elem_size=D, single_packet=False,
                        queue_num=pp['qassign'][ci])
                    if ci == 0:
                        nc.gpsimd.partition_broadcast(out_ap=xu[:], in_ap=xu_row[:])
                    for b in range(b0, b0 + nblk):
                        lc = int(CB[b]) - cb0
                        red = rp.tile([P, D], f32, tag="red")
                        nc.vector.tensor_reduce(
                            out=red[:],
                            in_=g[:, lc:lc + int(W[b]), :].rearrange("p c d -> p d c"),
                            axis=X, op=AO.add)
                        corr = rp.tile([P, D], f32, tag="corr")
                        nc.vector.scalar_tensor_tensor(
                            out=corr[:], in0=xu[:], scalar=npad[:, b:b + 1],
                            in1=red[:], op0=AO.mult, op1=AO.add)
                        if not last_hop:
                            nc.scalar.mul(out=nh[:, bs(b)], in_=corr[:],
                                          mul=nrm2[:, b:b + 1])
                            nc.sync.dma_start(out=ag_in[:, bs(b)],
                                              in_=nh[:, bs(b)])
                        else:
                            nc.scalar.mul(out=nh[:, bs(b)], in_=corr[:],
                                          mul=nrm[:, b:b + 1])
                            nc.sync.dma_start(out=t_r[:, bs(b)],
                                              in_=nh[:, bs(b)])
                            if b < NB - 16 and (b + 1) % GRP == 0:
                                ln_ffn_group(b + 1 - GRP)
                            elif b >= NB - 16 and (b + 1) % 4 == 0:
                                ln_ffn_group(b - 3, 4)

    nc.compile()
    return nc


def kernel(features, edge_src, edge_dst, w1, b1, w2, b2, gamma, beta):
    from concourse import bass_utils
    features = np.asarray(features, np.float32)
    edge_src = np.asarray(edge_src, np.int32)
    edge_dst = np.asarray(edge_dst, np.int32)
    N = features.shape[0]
    NPC = N // NC
    NB = NPC // P

    deg = np.bincount(edge_dst, minlength=N).astype(np.float32)
    norm = 1.0 / np.sqrt(np.maximum(deg, 1.0))

    import hashlib
    h = hashlib.sha1()
    h.update(edge_src.tobytes())
    h.update(edge_dst.tobytes())
    h.update(str(N).encode())
    key = h.hexdigest()
    if key not in _CACHE:
        pp = _preprocess(N, edge_src, edge_dst)
        ncb = _build(N, pp)
        _CACHE[key] = (pp, ncb)
    pp, ncb = _CACHE[key]

    # host-computed hop-1 table: norm*features in table-row order
    nf = norm[:, None] * features
    tab0_np = np.empty((N, D), np.float32)
    tab0_np[pp['trow']] = nf

    ident_np = np.eye(P, dtype=np.float32)
    w1T_np = np.ascontiguousarray(np.asarray(w1, np.float32).T)
    w2T_np = np.ascontiguousarray(np.asarray(w2, np.float32).T)
    b1_np = np.asarray(b1, np.float32).reshape(D, 1)
    b2b_np = np.tile(np.asarray(b2, np.float32)[None, :], (P, 1))
    gam_np = np.tile(np.asarray(gamma, np.float32)[None, :], (P, 1))
    bet_np = np.tile(np.asarray(beta, np.float32)[None, :], (P, 1))

    in_maps = []
    for k in range(NC):
        o = pp['order'][k]
        # position s = b*128+p; feat tile [p, b*64+d]
        fo = features[k * NPC + o].reshape(NB, P, D).transpose(1, 0, 2) \
            .reshape(P, NB * D).copy()
        no = norm[k * NPC + o].reshape(NB, P).T.copy()
        in_maps.append({
            "feat": fo, "tab0": tab0_np, "idx16": pp['idx16'][k],
            "normv": no, "norm2v": (no * no), "npadv": pp['negpad'][k],
            "ident": ident_np,
            "w1T": w1T_np, "w2T": w2T_np, "b1c": b1_np, "b2b": b2b_np,
            "gamb": gam_np, "betb": bet_np,
        })

    trace = os.environ.get("GCN_TRACE", "0") == "1"
    res = bass_utils.run_bass_kernel_spmd(ncb, in_maps, core_ids=list(range(NC)),
                                          trace=trace)
    if trace and res.exec_time_ns is not None:
        print(f"HW exec time: {res.exec_time_ns} ns")
    if trace and res.instructions_and_trace is not None:
        print(f"Trace path: {res.instructions_and_trace[1]}")

    out = np.empty((N, D), np.float32)
    r = np.empty((N, D), np.float32)
    for k in range(NC):
        o = res.results[k]["outp"].reshape(P, NB, D).transpose(1, 0, 2).reshape(NPC, D)
        rr = res.results[k]["routp"].reshape(P, NB, D).transpose(1, 0, 2).reshape(NPC, D)
        out[k * NPC + pp['order'][k]] = o
        r[k * NPC + pp['order'][k]] = rr
    return (out, r)


# revision 24
# speedup vs baseline: 1.0513x; 1.0261x over previous
"""GCN block (3-hop symmetric-normalized propagation + LN/FFN/residual) on 8 trn2 cores.

v4: identity-slot gather (no one-hots, no scatter matmuls).
  - Table per hop: [65536, 64] fp32 (256B rows) in DRAM. Signed int16 gather
    indices with the base biased to row 32768 address all 65536 rows
    (idx = row - 32768; the Q7 address math sign-extends correctly).
  - Nodes sharded 8 ways; per core, dst nodes are degree-sorted (desc) into
    64 blocks of 128. Block b gets W[b] = max in-block degree columns
    (shared schedule = max over cores). Edge e of dst (b,p) lands at
    gather slot [partition p, column CB[b]+j] -- directly in destination
    position. Pad slots point at row 65535 (idx +32767, also guarantees the
    per-call idx tail is non-negative so the uCode trim never fires).
  - Aggregation per block = ONE strided tensor_reduce over the block's
    columns (DVE), then a rank-1 pad correction (scalar_tensor_tensor with
    per-partition pad counts x table row 65535) and the norm scale on the
    scalar engine. GPSIMD does nothing but desc-gen, on 4 SWDGE queues.
  - Hop-1 table = norm*features, host-computed. Hops 2/3 tables via
    AllGather of each core's scaled block results (fp32, 2MB/core).
  - LN + FFN fp32, interleaved per 8-block group under the hop-3 stream.
"""
import sys
sys.path.insert(0, '/opt/trn_rl_repo')
import os
import numpy as np

NC = 8          # cores
P = 128         # partitions
D = 64          # feature dim
HOPS = 3
LN_EPS = 1e-5
MAXCOLS = 28    # max gather columns per call (ring-safe; small calls pipeline better)
PADVAL = 32767  # pad idx -> table row 65535

_CACHE = {}


def _preprocess(N, edge_src, edge_dst):
    NPC = N // NC          # nodes per core (8192)
    NB = NPC // P          # dst blocks per core (64)

    deg = np.bincount(edge_dst, minlength=N).astype(np.int64)

    NCHUNK = 4                                 # AllGather chunks per hop
    BPC = NB // NCHUNK                         # blocks per chunk (16)
    order = np.empty((NC, NPC), np.int64)      # position s -> local node
    pos_of = np.empty((NC, NPC), np.int64)     # local node -> position s
    trow = np.empty(N, np.int64)               # node -> table row
    degs_blk = np.empty((NC, NB, P), np.int64)
    for k in range(NC):
        d_loc = deg[k * NPC:(k + 1) * NPC]
        o = np.argsort(-d_loc, kind='stable')
        order[k] = o
        pos_of[k, o] = np.arange(NPC)
        s = np.arange(NPC)
        b, p = s // P, s % P
        trow[k * NPC + o] = k * NPC + p * NB + b
        degs_blk[k] = d_loc[o].reshape(NB, P)

    W = degs_blk.max(axis=(0, 2))              # [NB] shared schedule
    W = np.maximum(W, 1)
    # force a pad at (p=127, last col) for every core (tail-trim guard)
    tailmax = degs_blk[:, :, P - 1].max(axis=0)   # [NB]
    W[tailmax >= W] = tailmax[tailmax >= W] + 1
    CB = np.zeros(NB + 1, np.int64)
    np.cumsum(W, out=CB[1:])
    TC = int(CB[-1])
    assert W.max() <= 44, f"block needs {W.max()} cols > ring-safe 44"

    # greedy whole-block call grouping
    calls = []   # (b0, nblk, cb0, ncols)
    b = 0
    while b < NB:
        b0, cols = b, 0
        while b < NB and cols + W[b] <= MAXCOLS:
            cols += W[b]
            b += 1
        if b == b0:          # oversized single block
            cols = W[b]
            b += 1
        calls.append((b0, b - b0, int(CB[b0]), int(cols)))
    qload = [0, 0, 0, 0]
    qassign = []
    for (_, _, _, ncols) in calls:
        q = qload.index(min(qload))
        qassign.append(q)
        qload[q] += ncols

    idx_all = np.full((NC, TC * P), PADVAL, np.int64)
    s64 = edge_src.astype(np.int64)
    d64 = edge_dst.astype(np.int64)
    kd = d64 // NPC
    for k in range(NC):
        m = kd == k
        dpos = pos_of[k, d64[m] - k * NPC]
        src_r = trow[s64[m]] - N // 2
        # sort by (dst, ascending src row): column c of a block then holds
        # similar-quantile rows -> better HBM locality for the random reads
        o = np.lexsort((src_r, dpos))
        ds = dpos[o]
        starts = np.r_[0, np.flatnonzero(np.diff(ds)) + 1]
        runlen = np.diff(np.r_[starts, len(ds)])
        cc = np.arange(len(ds)) - np.repeat(starts, runlen)
        bb, pp_ = ds // P, ds % P
        col = CB[bb] + cc
        idx_all[k, col * P + pp_] = src_r[o]
    # wrapped int16 layout [i%16, i//16], replicated to 128 partitions
    idx16 = idx_all.reshape(NC, TC * 8, 16).transpose(0, 2, 1).astype(np.int16)
    idx16 = np.tile(idx16, (1, 8, 1))          # [NC, 128, TC*8]

    negpad = -(W[None, :, None] - degs_blk).astype(np.float32)  # [NC, NB, P]
    negpad = negpad.transpose(0, 2, 1).copy()                   # [NC, P, NB]

    return dict(NPC=NPC, NB=NB, TC=TC, W=W, CB=CB, calls=calls,
                qassign=qassign, idx16=idx16, negpad=negpad, order=order,
                trow=trow, NCHUNK=NCHUNK, BPC=BPC)


def _build(N, pp):
    from concourse import bass, bacc, tile, mybir
    NPC, NB, TC = pp['NPC'], pp['NB'], pp['TC']
    W, CB = pp['W'], pp['CB']
    f32, i16 = mybir.dt.float32, mybir.dt.int16
    AO = mybir.AluOpType
    X = mybir.AxisListType.X

    nc = bacc.Bacc("TRN2", target_bir_lowering=False, debug=False, num_devices=NC,
                   num_swdge_queues=4)
    t_feat = nc.dram_tensor("feat", [P, NB * D], f32, kind="ExternalInput")
    t_tab0 = nc.dram_tensor("tab0", [N, D], f32, kind="ExternalInput")
    t_idx = nc.dram_tensor("idx16", [P, TC * 8], i16, kind="ExternalInput")
    t_norm = nc.dram_tensor("normv", [P, NB], f32, kind="ExternalInput")
    t_norm2 = nc.dram_tensor("norm2v", [P, NB], f32, kind="ExternalInput")
    t_npad = nc.dram_tensor("npadv", [P, NB], f32, kind="ExternalInput")
    t_ident = nc.dram_tensor("ident", [P, P], f32, kind="ExternalInput")
    t_w1T = nc.dram_tensor("w1T", [D, D], f32, kind="ExternalInput")
    t_w2T = nc.dram_tensor("w2T", [D, D], f32, kind="ExternalInput")
    t_b1 = nc.dram_tensor("b1c", [D, 1], f32, kind="ExternalInput")
    t_b2 = nc.dram_tensor("b2b", [P, D], f32, kind="ExternalInput")
    t_gam = nc.dram_tensor("gamb", [P, D], f32, kind="ExternalInput")
    t_bet = nc.dram_tensor("betb", [P, D], f32, kind="ExternalInput")
    t_out = nc.dram_tensor("outp", [P, NB * D], f32, kind="ExternalOutput")
    t_r = nc.dram_tensor("routp", [P, NB * D], f32, kind="ExternalOutput")

    with tile.TileContext(nc) as tc:
        with tc.tile_pool(name="const", bufs=1) as cp, \
             tc.tile_pool(name="work", bufs=1) as wp, \
             tc.tile_pool(name="g", bufs=12) as gp, \
             tc.tile_pool(name="r", bufs=6) as rp, \
             tc.tile_pool(name="oh", bufs=2) as op_, \
             tc.tile_pool(name="ps", bufs=2, space="PSUM") as ps, \
             tc.tile_pool(name="dram", bufs=1, space="DRAM") as dr:

            idx16 = cp.tile([P, TC * 8], i16)
            c3 = pp['calls'][2][2] + pp['calls'][2][3]   # cols thru call 2
            nc.sync.dma_start(out=idx16[:, :c3 * 8], in_=t_idx[:, :c3 * 8])
            nc.sync.dma_start(out=idx16[:, c3 * 8:], in_=t_idx[:, c3 * 8:])
            feat = cp.tile([P, NB * D], f32)
            nrm = cp.tile([P, NB], f32)
            nrm2 = cp.tile([P, NB], f32)
            npad = cp.tile([P, NB], f32)
            idn = cp.tile([P, P], f32)
            w1T = cp.tile([D, D], f32)
            w2T = cp.tile([D, D], f32)
            b1 = cp.tile([D, 1], f32)
            b2b = cp.tile([P, D], f32)
            gmb = cp.tile([P, D], f32)
            btb = cp.tile([P, D], f32)
            for tl, th in ((nrm, t_norm), (nrm2, t_norm2), (npad, t_npad),
                           (feat, t_feat), (idn, t_ident), (w1T, t_w1T),
                           (w2T, t_w2T), (b1, t_b1), (b2b, t_b2),
                           (gmb, t_gam), (btb, t_bet)):
                nc.sync.dma_start(out=tl[:], in_=th[:])

            nh = wp.tile([P, NB * D], f32)
            ag_in = dr.tile([P, NB * D], f32)
            tables = [t_tab0]
            for hh in range(1, HOPS):
                tbl = dr.tile([N, D], f32, addr_space="Shared", tag=f"table{hh}")
                tables.append(tbl)

            def bs(b):
                return slice(b * D, (b + 1) * D)

            GRP = 8
            out_own = wp.tile([P, NB * D], f32)

            def ln_ffn_group(b0, n=GRP):
                """LN + FFN + residuals + store for blocks b0..b0+n-1."""
                sl = slice(b0 * D, (b0 + n) * D)
                r3 = nh[:, sl].rearrange("p (b d) -> p b d", d=D)
                xc = wp.tile([P, GRP * D], f32, tag="xc", bufs=2)
                xc3 = xc[:, :n * D].rearrange("p (b d) -> p b d", d=D)
                sq = wp.tile([P, GRP * D], f32, tag="sq", bufs=2)
                sq3 = sq[:, :n * D].rearrange("p (b d) -> p b d", d=D)
                mu = wp.tile([P, GRP], f32, tag="mu", bufs=2)
                ssq = wp.tile([P, GRP], f32, tag="ssq", bufs=2)
                rstd = wp.tile([P, GRP], f32, tag="rstd", bufs=2)
                nc.vector.tensor_reduce(out=mu[:, :n], in_=r3, axis=X, op=AO.add)
                nc.vector.tensor_scalar(out=mu[:, :n], in0=mu[:, :n], scalar1=1.0 / D,
                                        scalar2=None, op0=AO.mult)
                nc.vector.tensor_tensor(out=xc3, in0=r3,
                                        in1=mu[:, :n].rearrange("p (b o) -> p b o", o=1).to_broadcast([P, n, D]),
                                        op=AO.subtract)
                nc.vector.tensor_tensor(out=sq3, in0=xc3, in1=xc3, op=AO.mult)
                nc.vector.tensor_reduce(out=ssq[:, :n], in_=sq3, axis=X, op=AO.add)
                nc.vector.tensor_scalar(out=ssq[:, :n], in0=ssq[:, :n], scalar1=1.0 / D,
                                        scalar2=None, op0=AO.mult)
                nc.vector.tensor_scalar(out=ssq[:, :n], in0=ssq[:, :n], scalar1=LN_EPS,
                                        scalar2=None, op0=AO.add)
                nc.scalar.activation(out=ssq[:, :n], in_=ssq[:, :n],
                                     func=mybir.ActivationFunctionType.Sqrt)
                nc.vector.reciprocal(rstd[:, :n], ssq[:, :n])
                nc.vector.tensor_tensor(out=xc3, in0=xc3,
                                        in1=rstd[:, :n].rearrange("p (b o) -> p b o", o=1).to_broadcast([P, n, D]),
                                        op=AO.mult)
                nc.vector.tensor_tensor(out=xc3, in0=xc3,
                                        in1=gmb[:].rearrange("p (o d) -> p o d", o=1).to_broadcast([P, n, D]),
                                        op=AO.mult)
                nc.vector.tensor_tensor(out=xc3, in0=xc3,
                                        in1=btb[:].rearrange("p (o d) -> p o d", o=1).to_broadcast([P, n, D]),
                                        op=AO.add)
                for i in range(n):
                    b = b0 + i
                    xT_ps = ps.tile([D, P], f32, tag="tr", space="PSUM")
                    nc.tensor.transpose(out=xT_ps[:], in_=xc[:, i * D:(i + 1) * D],
                                        identity=idn[:])
                    xT = op_.tile([D, P], f32, tag="xT")
                    nc.scalar.copy(xT[:], xT_ps[:])
                    h1_ps = ps.tile([D, P], f32, tag="h1", space="PSUM")
                    nc.tensor.matmul(out=h1_ps[:], lhsT=w1T[:], rhs=xT[:],
                                     start=True, stop=True)
                    h1 = op_.tile([D, P], f32, tag="h1s")
                    nc.scalar.activation(out=h1[:], in_=h1_ps[:],
                                         func=mybir.ActivationFunctionType.Relu,
                                         bias=b1[:, 0:1])
                    ff_ps = ps.tile([P, D], f32, tag="ff", space="PSUM")
                    nc.tensor.matmul(out=ff_ps[:], lhsT=h1[:], rhs=w2T[:],
                                     start=True, stop=True)
                    nc.vector.tensor_tensor(out=out_own[:, bs(b)], in0=ff_ps[:],
                                            in1=nh[:, bs(b)], op=AO.add)
                o3 = out_own[:, sl].rearrange("p (b d) -> p b d", d=D)
                nc.vector.tensor_tensor(out=o3, in0=o3,
                                        in1=feat[:, sl].rearrange("p (b d) -> p b d", d=D),
                                        op=AO.add)
                nc.vector.tensor_tensor(out=o3, in0=o3,
                                        in1=b2b[:].rearrange("p (o d) -> p o d", o=1).to_broadcast([P, n, D]),
                                        op=AO.add)
                nc.sync.dma_start(out=t_out[:, sl], in_=out_own[:, sl])

            rg = [list(range(NC))]
            for hop in range(1, HOPS + 1):
                table = tables[hop - 1]
                if hop > 1:
                    nc.gpsimd.collective_compute("AllGather", AO.bypass,
                                                 replica_groups=rg,
                                                 ins=[ag_in[:]], outs=[table[:]])
                # pad-correction row (table row N-1), replicated to 128 parts
                xu_row = rp.tile([1, D], f32, tag="xur")
                nc.sync.dma_start(out=xu_row[:], in_=table[N - 1:N, :])
                xu = rp.tile([P, D], f32, tag="xu")

                last_hop = hop == HOPS
                for ci, (b0, nblk, cb0, ncols) in enumerate(pp['calls']):
                    g = gp.tile([P, max(MAXCOLS, int(pp["W"].max())), D], f32, tag="g")
                    nc.gpsimd.dma_gather(
                        out_ap=g[:, :ncols, :], in_ap=table[N // 2:, :],
                        idxs_ap=idx16[:, cb0 * 8:(cb0 + ncols) * 8],
                        num_idxs=ncols * P, num_idxs_reg=ncols * P,
                        elem_size=D, single_packet=False,
                        queue_num=pp['qassign'][ci])
                    if ci == 0:
                        nc.gpsimd.partition_broadcast(out_ap=xu[:], in_ap=xu_row[:])
                    for b in range(b0, b0 + nblk):
                        lc = int(CB[b]) - cb0
                        red = rp.tile([P, D], f32, tag="red")
                        nc.vector.tensor_reduce(
                            out=red[:],
                            in_=g[:, lc:lc + int(W[b]), :].rearrange("p c d -> p d c"),
                            axis=X, op=AO.add)
                        corr = rp.tile([P, D], f32, tag="corr")
                        nc.vector.scalar_tensor_tensor(
                            out=corr[:], in0=xu[:], scalar=npad[:, b:b + 1],
                            in1=red[:], op0=AO.mult, op1=AO.add)
                        if not last_hop:
                            nc.scalar.mul(out=nh[:, bs(b)], in_=corr[:],
                                          mul=nrm2[:, b:b + 1])
                            nc.sync.dma_start(out=ag_in[:, bs(b)],
                                              in_=nh[:, bs(b)])
                        else:
                            nc.scalar.mul(out=nh[:, bs(b)], in_=corr[:],
                                          mul=nrm[:, b:b + 1])
                            nc.sync.dma_start(out=t_r[:, bs(b)],
                                              in_=nh[:, bs(b)])
                            if b < NB - 16 and (b + 1) % GRP == 0:
                                ln_ffn_group(b + 1 - GRP)
                            elif b >= NB - 16 and (b + 1) % 4 == 0:
                                ln_ffn_group(b - 3, 4)

    nc.compile()
    return nc


def kernel(features, edge_src, edge_dst, w1, b1, w2, b2, gamma, beta):
    from concourse import bass_utils
    features = np.asarray(features, np.float32)
    edge_src = np.asarray(edge_src, np.int32)
    edge_dst = np.asarray(edge_dst, np.int32)
    N = features.shape[0]
    NPC = N // NC
    NB = NPC // P

    deg = np.bincount(edge_dst, minlength=N).astype(np.float32)
    norm = 1.0 / np.sqrt(np.maximum(deg, 1.0))

    import hashlib
    h = hashlib.sha1()
    h.update(edge_src.tobytes())
    h.update(edge_dst.tobytes())
    h.update(str(N).encode())
    key = h.hexdigest()
    if key not in _CACHE:
        pp = _preprocess(N, edge_src, edge_dst)
        ncb = _build(N, pp)
        _CACHE[key] = (pp, ncb)
    pp, ncb = _CACHE[key]

    # host-computed hop-1 table: norm*features in table-row order
    nf = norm[:, None] * features
    tab0_np = np.empty((N, D), np.float32)
    tab0_np[pp['trow']] = nf

    ident_np = np.eye(P, dtype=np.float32)
    w1T_np = np.ascontiguousarray(np.asarray(w1, np.float32).T)
    w2T_np = np.ascontiguousarray(np.asarray(w2, np.float32).T)
    b1_np = np.asarray(b1, np.float32).reshape(D, 1)
    b2b_np = np.tile(np.asarray(b2, np.float32)[None, :], (P, 1))
    gam_np = np.tile(np.asarray(gamma, np.float32)[None, :], (P, 1))
    bet_np = np.tile(np.asarray(beta, np.float32)[None, :], (P, 1))

    in_maps = []
    for k in range(NC):
        o = pp['order'][k]
        # position s = b*128+p; feat tile [p, b*64+d]
        fo = features[k * NPC + o].reshape(NB, P, D).transpose(1, 0, 2) \
            .reshape(P, NB * D).copy()
        no = norm[k * NPC + o].reshape(NB, P).T.copy()
        in_maps.append({
            "feat": fo, "tab0": tab0_np, "idx16": pp['idx16'][k],
            "normv": no, "norm2v": (no * no), "npadv": pp['negpad'][k],
            "ident": ident_np,
            "w1T": w1T_np, "w2T": w2T_np, "b1c": b1_np, "b2b": b2b_np,
            "gamb": gam_np, "betb": bet_np,
        })

    trace = os.environ.get("GCN_TRACE", "0") == "1"
    res = bass_utils.run_bass_kernel_spmd(ncb, in_maps, core_ids=list(range(NC)),
                                          trace=trace)
    if trace and res.exec_time_ns is not None:
        print(f"HW exec time: {res.exec_time_ns} ns")
    if trace and res.instructions_and_trace is not None:
        print(f"Trace path: {res.instructions_and_trace[1]}")

    out = np.empty((N, D), np.float32)
    r = np.empty((N, D), np.float32)
    for k in range(NC):
        o = res.results[k]["outp"].reshape(P, NB, D).transpose(1, 0, 2).reshape(NPC, D)
        rr = res.results[k]["routp"].reshape(P, NB, D).transpose(1, 0, 2).reshape(NPC, D)
        out[k * NPC + pp['order'][k]] = o
        r[k * NPC + pp['order'][k]] = rr
    return (out, r)
